# revision 7
# baseline (speedup 1.0000x reference)
"""Trainium2 Bass kernel for nn_CausalSelfAttention_17188459119385.

Sharding: 8 cores = batch (2) x KV-head groups (4).  Core c handles batch
c//4 and KV head c%4 (with its 4 grouped query heads).  Each core computes
a partial output y_part = attn_out @ w_o[rows of its heads]; the host sums
the 4 partials per batch and adds b_o.

v2 dataflow (all matmul operands bf16, fp32 PSUM accumulation):
  - x^T is pre-transposed and pre-tiled on the HOST into the exact SBUF
    layout [128, a, t] (contiguous 8KB DMA lines), so the PE does zero
    transposes and the projections start as soon as the first t-chunk of
    x^T lands.
  - Per t-chunk ti: K^T chunk (dup'd on partitions 0:64 / 64:128 so both
    halves of a head pair have base-aligned operands), V chunk (natural
    [s, d] with a ones-column so the PV matmul emits softmax row-sums for
    free), Q^T chunk, then attention for that chunk.
  - Scores are computed FULL-width (512 cols) per s-block so exp can be
    batched: s-blocks are grouped G=3 per PSUM allocation and a single
    ACT instruction exps the whole group (cuts ACT instruction count
    ~3x; the per-instruction overhead is ~350 cycles).  Causal masking
    inside diagonal 128-blocks is done by ACCUMULATING a -1e9 strictly-
    lower-triangular constant via a second matmul into the score PSUM
    before the exp - no vector-engine op on the ACT->PV critical path.
    Columns left of the diagonal (t < s-block start) hold live-but-
    unused scores; the PV matmul simply skips them (j0 pruning).
  - O~^T accumulates over s-blocks in PSUM; row 64 is the rowsum.  The
    reciprocal rowsum (fast-approx DVE, read straight from PSUM) is
    broadcast across partitions on GPSIMD and multiplied in during the
    PSUM->SBUF copy; odd heads' tiles are DMA-shifted to partitions
    64:128.
  - y^T = w_o^T O^T streams out per 128-row chunk in bf16 (the host
    accumulates partials in fp32); y(ti-1) chunks are emitted between
    attention heads of ti to fill PE slack while ACT works.
"""

import sys

if "/opt/trn_rl_repo" not in sys.path:
    sys.path.insert(0, "/opt/trn_rl_repo")

import numpy as np
import ml_dtypes

B, T, C = 2, 2048, 1024
NKV, G_HEADS, D = 4, 4, 64     # kv heads, q-heads per kv head, head dim
QD = G_HEADS * D               # 256: q-feature width per core
P = 128
TCH = 512                      # t-chunk (matmul moving width)
NT = T // TCH                  # 4
NCC = C // P                   # 8 contraction chunks
NS = T // P                    # 16 s-blocks
GRP = 3                        # s-blocks per exp batch (3 PSUM banks)
BF16 = ml_dtypes.bfloat16

_CACHE = {}


def _build_nc():
    import concourse.mybir as mybir
    from concourse import bacc
    from concourse.tile import TileContext

    dt = mybir.dt
    AF = mybir.ActivationFunctionType

    nc = bacc.Bacc("TRN2", target_bir_lowering=False, debug=False)

    # host-pre-tiled inputs: partition-major, contiguous per-partition rows
    xt_d = nc.dram_tensor("xt", [P, NT * NCC * TCH], dt.bfloat16,
                          kind="ExternalInput")
    wq_d = nc.dram_tensor("wq", [P, NCC * QD], dt.bfloat16,
                          kind="ExternalInput")
    wk_d = nc.dram_tensor("wk", [P, NCC * P], dt.bfloat16,
                          kind="ExternalInput")
    wv_d = nc.dram_tensor("wv", [P, NCC * D], dt.bfloat16,
                          kind="ExternalInput")
    wo_d = nc.dram_tensor("wo", [P, 2 * C], dt.bfloat16,
                          kind="ExternalInput")
    bq_d = nc.dram_tensor("bq", [P, 2], dt.float32, kind="ExternalInput")
    bk_d = nc.dram_tensor("bk", [P, 1], dt.float32, kind="ExternalInput")
    bvr_d = nc.dram_tensor("bvr", [P, D], dt.float32, kind="ExternalInput")
    tri_d = nc.dram_tensor("tri", [P, P], dt.bfloat16, kind="ExternalInput")
    idin = nc.dram_tensor("idin", [P, P], dt.bfloat16, kind="ExternalInput")
    yt = nc.dram_tensor("yt", [NT * 8 * P, TCH], dt.bfloat16,
                        kind="ExternalOutput")

    with TileContext(nc) as tc:
        with (
            tc.tile_pool(name="const", bufs=1) as cpool,
            tc.tile_pool(name="xt", bufs=NT) as xtpool,
            tc.tile_pool(name="kt", bufs=1) as ktpool,
            tc.tile_pool(name="qt", bufs=2) as qtpool,
            tc.tile_pool(name="v", bufs=1) as vpool,
            tc.tile_pool(name="ot", bufs=2) as otpool,
            tc.tile_pool(name="p", bufs=3) as ppool,
            tc.tile_pool(name="r", bufs=4) as rpool,
            tc.tile_pool(name="rbs", bufs=4) as rbspool,
            tc.tile_pool(name="y", bufs=4) as ypool,
            tc.tile_pool(name="otmp", bufs=4) as otmp,
            tc.tile_pool(name="s", bufs=2, space="PSUM") as spool,
            tc.tile_pool(name="o", bufs=1, space="PSUM") as ops_,
            tc.tile_pool(name="mm", bufs=1, space="PSUM") as mmps,
        ):
            # ---- input DMAs (xt per t-chunk; weights; constants) ----
            xts = []
            for ti in range(NT):
                xs = xtpool.tile([P, NCC, TCH], dt.bfloat16, tag="xt",
                                 name=f"xt{ti}")
                nc.sync.dma_start(
                    xs[:], xt_d[:, ti * NCC * TCH:(ti + 1) * NCC * TCH])
                xts.append(xs)
            wq_sb = cpool.tile([P, NCC, QD], dt.bfloat16, tag="wq")
            nc.sync.dma_start(wq_sb[:], wq_d[:])
            wk_sb = cpool.tile([P, NCC, P], dt.bfloat16, tag="wk")
            nc.sync.dma_start(wk_sb[:], wk_d[:])
            wv_sb = cpool.tile([P, NCC, D], dt.bfloat16, tag="wv")
            nc.sync.dma_start(wv_sb[:], wv_d[:])
            wo_sb = cpool.tile([P, 2, C], dt.bfloat16, tag="wo")
            nc.sync.dma_start(wo_sb[:], wo_d[:])
            bq_sb = cpool.tile([P, 2], dt.float32, tag="bq")
            nc.sync.dma_start(bq_sb[:], bq_d[:])
            bk_sb = cpool.tile([P, 1], dt.float32, tag="bk")
            nc.sync.dma_start(bk_sb[:], bk_d[:])
            bvr_sb = cpool.tile([P, D], dt.float32, tag="bvr")
            nc.sync.dma_start(bvr_sb[:], bvr_d[:])
            tri_sb = cpool.tile([P, P], dt.bfloat16, tag="tri")
            nc.sync.dma_start(tri_sb[:], tri_d[:])
            ident = cpool.tile([P, P], dt.bfloat16, tag="ident")
            nc.sync.dma_start(ident[:], idin[:])

            Kt = ktpool.tile([P, T], dt.bfloat16, tag="kt")
            Qt = [qtpool.tile([P, T], dt.bfloat16, tag="qt", name=f"qt{i}")
                  for i in range(2)]
            Vb = vpool.tile([P, NS, D + 1], dt.bfloat16, tag="v")
            nc.gpsimd.memset(Vb[:], 1.0)
            Ot = [otpool.tile([P, T], dt.bfloat16, tag="ot", name=f"ot{i}")
                  for i in range(2)]

            def y_chunk(ti, ec):
                t0 = ti * TCH
                y_ps = mmps.tile([P, TCH], dt.float32, tag="mm")
                for dc in range(2):
                    nc.tensor.matmul(
                        y_ps[:], wo_sb[:, dc, ec * P:(ec + 1) * P],
                        Ot[dc][:, t0:t0 + TCH],
                        start=(dc == 0), stop=(dc == 1),
                    )
                y_sb = ypool.tile([P, TCH], dt.bfloat16, tag="y")
                nc.vector.tensor_copy(y_sb[:], y_ps[:])
                blk = (ti * 8 + ec) * P
                nc.sync.dma_start(yt[blk:blk + P, :], y_sb[:])

            for ti in range(NT):
                t0 = ti * TCH
                # ---- K^T chunk (dup'd across partition halves) ----
                ps = mmps.tile([P, TCH], dt.float32, tag="mm")
                for a in range(NCC):
                    nc.tensor.matmul(
                        ps[:], wk_sb[:, a, :], xts[ti][:, a, :],
                        start=(a == 0), stop=(a == NCC - 1),
                    )
                nc.vector.tensor_scalar_add(
                    Kt[:, t0:t0 + TCH], ps[:], bk_sb[:, 0:1])
                # ---- V chunks (natural [s, d]) ----
                for sj in range(4):
                    si = ti * 4 + sj
                    psv = mmps.tile([P, TCH], dt.float32, tag="mm")
                    for a in range(NCC):
                        nc.tensor.matmul(
                            psv[:, 0:D],
                            xts[ti][:, a, sj * P:(sj + 1) * P],
                            wv_sb[:, a, :],
                            start=(a == 0), stop=(a == NCC - 1),
                        )
                    nc.vector.tensor_add(Vb[:, si, 0:D], psv[:, 0:D],
                                         bvr_sb[:])
                # ---- Q^T chunk ----
                for qc in range(2):
                    psq = mmps.tile([P, TCH], dt.float32, tag="mm")
                    for a in range(NCC):
                        nc.tensor.matmul(
                            psq[:], wq_sb[:, a, qc * P:(qc + 1) * P],
                            xts[ti][:, a, :],
                            start=(a == 0), stop=(a == NCC - 1),
                        )
                    nc.vector.tensor_scalar_add(
                        Qt[qc][:, t0:t0 + TCH], psq[:], bq_sb[:, qc:qc + 1])

                # ---- attention for this t-chunk ----
                nsb = (t0 + TCH) // P
                for hi, h in enumerate((1, 0, 3, 2)):
                    qc, qr = divmod(h, 2)
                    q_ap = Qt[qc][qr * D:(qr + 1) * D, t0:t0 + TCH]
                    o_ps = ops_.tile([D + 1, TCH], dt.float32, tag="o")
                    sb = 0
                    while sb < nsb:
                        g = min(GRP, nsb - sb)
                        sgrp = spool.tile([P, GRP, TCH], dt.float32, tag="s")
                        for j in range(g):
                            s0 = (sb + j) * P
                            diag = s0 >= t0
                            nc.tensor.matmul(
                                sgrp[:, j, :],
                                Kt[qr * D:(qr + 1) * D, s0:s0 + P],
                                q_ap,
                                start=True, stop=not diag,
                                skip_group_check=diag,
                            )
                            if diag:
                                j0 = s0 - t0
                                nc.tensor.matmul(
                                    sgrp[:, j, j0:j0 + P],
                                    ident[:], tri_sb[:],
                                    start=False, stop=True,
                                    skip_group_check=True,
                                )
                        pt = ppool.tile([P, GRP, TCH], dt.bfloat16, tag="p")
                        nc.scalar.activation(
                            pt[:, 0:g, :], sgrp[:, 0:g, :], AF.Exp,
                            scale=0.125)
                        for j in range(g):
                            s0 = (sb + j) * P
                            j0 = max(s0 - t0, 0)
                            nc.tensor.matmul(
                                o_ps[:, j0:], Vb[:, sb + j, :],
                                pt[:, j, j0:],
                                start=(sb + j == 0),
                                stop=(sb + j == nsb - 1),
                            )
                        sb += g
                    rs = rpool.tile([1, TCH], dt.float32, tag="rs")
                    nc.vector.tensor_copy(rs[:], o_ps[D:D + 1, :])
                    rr = rpool.tile([1, TCH], dt.float32, tag="rr")
                    nc.vector.reciprocal_approx_fast(rr[:], rs[:])
                    rb_sb = rbspool.tile([D, TCH], dt.float32, tag="rbs")
                    nc.gpsimd.partition_broadcast(rb_sb[:], rr[:])
                    if qr == 0:
                        nc.vector.tensor_mul(
                            Ot[qc][0:D, t0:t0 + TCH], o_ps[0:D, :], rb_sb[:])
                    else:
                        ott = otmp.tile([D, TCH], dt.bfloat16, tag="ott")
                        nc.vector.tensor_mul(ott[:], o_ps[0:D, :], rb_sb[:])
                        nc.sync.dma_start(
                            Ot[qc][D:2 * D, t0:t0 + TCH], ott[:])
                    # weave previous chunk's output projection between heads
                    if ti > 0:
                        y_chunk(ti - 1, 2 * hi)
                        y_chunk(ti - 1, 2 * hi + 1)
            for ec in range(8):
                y_chunk(NT - 1, ec)

    nc.compile()
    return nc


def get_nc():
    if "nc" not in _CACHE:
        _CACHE["nc"] = _build_nc()
    return _CACHE["nc"]


def _tile_pm(arr, ncc=NCC):
    """[ncc*128, F] -> partition-major [128, ncc*F] (contiguous rows)."""
    n, f = arr.shape
    assert n == ncc * P
    return np.ascontiguousarray(
        arr.reshape(ncc, P, f).transpose(1, 0, 2).reshape(P, ncc * f))


def make_in_maps(x, w_q, b_q, w_k, b_k, w_v, b_v, w_o, b_o):
    """Host-side sharding: per-core input maps for cores 0..7."""
    tri = np.where(
        np.arange(P)[:, None] > np.arange(P)[None, :], -1e9, 0.0
    ).astype(BF16)
    ident = np.eye(P, dtype=np.float32).astype(BF16)
    in_maps = []
    xts = {}
    for b in range(B):
        # x[b]^T tiled: cols ordered (ti, a, t)
        xT = np.ascontiguousarray(np.asarray(x[b]).T).astype(BF16)  # [C, T]
        xts[b] = np.ascontiguousarray(
            xT.reshape(NCC, P, NT, TCH).transpose(1, 2, 0, 3).reshape(
                P, NT * NCC * TCH))
    for c in range(8):
        b, kv = divmod(c, NKV)
        q0 = kv * QD
        wkk = np.concatenate([w_k[:, kv * D:(kv + 1) * D]] * 2, axis=1)
        in_maps.append({
            "xt": xts[b],
            "wq": _tile_pm(np.asarray(w_q[:, q0:q0 + QD]).astype(BF16)),
            "wk": _tile_pm(np.asarray(wkk).astype(BF16)),
            "wv": _tile_pm(
                np.asarray(w_v[:, kv * D:(kv + 1) * D]).astype(BF16)),
            "wo": np.ascontiguousarray(
                np.asarray(w_o[q0:q0 + QD, :]).astype(BF16).reshape(
                    2, P, C).transpose(1, 0, 2).reshape(P, 2 * C)),
            "bq": np.ascontiguousarray(
                np.asarray(b_q[q0:q0 + QD]).astype(np.float32).reshape(
                    2, P).T),
            "bk": np.tile(np.asarray(b_k[kv * D:(kv + 1) * D]).astype(
                np.float32), 2).reshape(P, 1),
            "bvr": np.tile(np.asarray(b_v[kv * D:(kv + 1) * D]).astype(
                np.float32)[None, :], (P, 1)),
            "tri": tri,
            "idin": ident,
        })
    return in_maps


def unshard(results, b_o):
    """Sum per-core partial outputs into the full [B, T, C] fp32 output."""
    out = np.zeros((B, T, C), np.float32)
    for c in range(8):
        b = c // NKV
        arr = results[c]["yt"].astype(np.float32)  # [NT*8*128, 512]
        # rows blk=(ti*8+ec)*128+p hold y^T[ec*128+p, ti*512+t]
        ypart_t = arr.reshape(NT, C, TCH).transpose(1, 0, 2).reshape(C, T)
        out[b] += ypart_t.T
    out += np.asarray(b_o).astype(np.float32)[None, None, :]
    return out


def kernel(x, w_q, b_q, w_k, b_k, w_v, b_v, w_o, b_o):
    from concourse.bass_utils import run_bass_kernel_spmd

    x = np.asarray(x)
    nc = get_nc()
    in_maps = make_in_maps(x, np.asarray(w_q), np.asarray(b_q),
                           np.asarray(w_k), np.asarray(b_k),
                           np.asarray(w_v), np.asarray(b_v),
                           np.asarray(w_o), np.asarray(b_o))
    res = run_bass_kernel_spmd(nc, in_maps, list(range(8)))
    return unshard(res.results, b_o)


# revision 11
# speedup vs baseline: 1.2223x; 1.2223x over previous
"""Trainium2 Bass kernel for nn_CausalSelfAttention_17188459119385.

Sharding: 8 cores = batch (2) x KV-head groups (4).  Core c handles batch
c//4 and KV head c%4 (with its 4 grouped query heads).  Each core computes
a partial output y_part = attn_out @ w_o[rows of its heads]; the host sums
the 4 partials per batch and adds b_o.

v2 dataflow (all matmul operands bf16, fp32 PSUM accumulation):
  - x^T is pre-transposed and pre-tiled on the HOST into the exact SBUF
    layout [128, a, t] (contiguous 8KB DMA lines), so the PE does zero
    transposes and the projections start as soon as the first t-chunk of
    x^T lands.
  - Per t-chunk ti: K^T chunk (dup'd on partitions 0:64 / 64:128 so both
    halves of a head pair have base-aligned operands), V chunk (natural
    [s, d] with a ones-column so the PV matmul emits softmax row-sums for
    free), Q^T chunk, then attention for that chunk.
  - Scores are computed FULL-width (512 cols) per s-block so exp can be
    batched: s-blocks are grouped G=3 per PSUM allocation and a single
    ACT instruction exps the whole group (cuts ACT instruction count
    ~3x; the per-instruction overhead is ~350 cycles).  Causal masking
    inside diagonal 128-blocks is done by ACCUMULATING a -1e9 strictly-
    lower-triangular constant via a second matmul into the score PSUM
    before the exp - no vector-engine op on the ACT->PV critical path.
    Columns left of the diagonal (t < s-block start) hold live-but-
    unused scores; the PV matmul simply skips them (j0 pruning).
  - O~^T accumulates over s-blocks in PSUM; row 64 is the rowsum.  The
    reciprocal rowsum (fast-approx DVE, read straight from PSUM) is
    broadcast across partitions on GPSIMD and multiplied in during the
    PSUM->SBUF copy; odd heads' tiles are DMA-shifted to partitions
    64:128.
  - y^T = w_o^T O^T streams out per 128-row chunk in bf16 (the host
    accumulates partials in fp32); y(ti-1) chunks are emitted between
    attention heads of ti to fill PE slack while ACT works.
"""

import sys

if "/opt/trn_rl_repo" not in sys.path:
    sys.path.insert(0, "/opt/trn_rl_repo")

import numpy as np
import ml_dtypes

B, T, C = 2, 2048, 1024
NKV, G_HEADS, D = 4, 4, 64     # kv heads, q-heads per kv head, head dim
QD = G_HEADS * D               # 256: q-feature width per core
P = 128
TCH = 512                      # t-chunk (matmul moving width)
NT = T // TCH                  # 4
NCC = C // P                   # 8 contraction chunks
NS = T // P                    # 16 s-blocks
GRP = 2                        # s-blocks per exp batch (2 PSUM banks)
BF16 = ml_dtypes.bfloat16

_CACHE = {}


def _build_nc():
    import concourse.mybir as mybir
    from concourse import bacc
    from concourse.tile import TileContext

    dt = mybir.dt
    AF = mybir.ActivationFunctionType

    nc = bacc.Bacc("TRN2", target_bir_lowering=False, debug=False)

    # host-pre-tiled inputs: partition-major, contiguous per-partition rows
    xt_d = nc.dram_tensor("xt", [P, NT * NCC * TCH], dt.bfloat16,
                          kind="ExternalInput")
    wq_d = nc.dram_tensor("wq", [P, NCC * QD], dt.bfloat16,
                          kind="ExternalInput")
    wk_d = nc.dram_tensor("wk", [P, NCC * P], dt.bfloat16,
                          kind="ExternalInput")
    wv_d = nc.dram_tensor("wv", [P, NCC * D], dt.bfloat16,
                          kind="ExternalInput")
    wo_d = nc.dram_tensor("wo", [P, 2 * C], dt.bfloat16,
                          kind="ExternalInput")
    bq_d = nc.dram_tensor("bq", [P, 2], dt.float32, kind="ExternalInput")
    bk_d = nc.dram_tensor("bk", [P, 1], dt.float32, kind="ExternalInput")
    bvr_d = nc.dram_tensor("bvr", [P, D], dt.float32, kind="ExternalInput")
    tri_d = nc.dram_tensor("tri", [P, P], dt.bfloat16, kind="ExternalInput")
    idin = nc.dram_tensor("idin", [P, P], dt.bfloat16, kind="ExternalInput")
    yt = nc.dram_tensor("yt", [NT * 8 * P, TCH], dt.bfloat16,
                        kind="ExternalOutput")

    with TileContext(nc) as tc:
        with (
            tc.tile_pool(name="const", bufs=1) as cpool,
            tc.tile_pool(name="xt", bufs=NT) as xtpool,
            tc.tile_pool(name="kt", bufs=1) as ktpool,
            tc.tile_pool(name="qt", bufs=2) as qtpool,
            tc.tile_pool(name="v", bufs=1) as vpool,
            tc.tile_pool(name="ot", bufs=2) as otpool,
            tc.tile_pool(name="p", bufs=3) as ppool,
            tc.tile_pool(name="r", bufs=4) as rpool,
            tc.tile_pool(name="rbs", bufs=4) as rbspool,
            tc.tile_pool(name="y", bufs=4) as ypool,
            tc.tile_pool(name="otmp", bufs=4) as otmp,
            tc.tile_pool(name="s", bufs=2, space="PSUM") as spool,
            tc.tile_pool(name="o", bufs=2, space="PSUM") as ops_,
            tc.tile_pool(name="mm", bufs=2, space="PSUM") as mmps,
        ):
            # ---- input DMAs: small weights first (sync q), x^T chunks
            # split across the sync and scalar hw DMA queues ----
            wk_sb = cpool.tile([P, NCC, P], dt.bfloat16, tag="wk")
            nc.sync.dma_start(wk_sb[:], wk_d[:])
            bk_sb = cpool.tile([P, 1], dt.float32, tag="bk")
            nc.sync.dma_start(bk_sb[:], bk_d[:])
            bq_sb = cpool.tile([P, 2], dt.float32, tag="bq")
            nc.sync.dma_start(bq_sb[:], bq_d[:])
            bvr_sb = cpool.tile([P, D], dt.float32, tag="bvr")
            nc.sync.dma_start(bvr_sb[:], bvr_d[:])
            tri_sb = cpool.tile([P, P], dt.bfloat16, tag="tri")
            nc.sync.dma_start(tri_sb[:], tri_d[:])
            ident = cpool.tile([P, P], dt.bfloat16, tag="ident")
            nc.sync.dma_start(ident[:], idin[:])
            wq_sb = cpool.tile([P, NCC, QD], dt.bfloat16, tag="wq")
            nc.scalar.dma_start(wq_sb[:], wq_d[:])
            wv_sb = cpool.tile([P, NCC, D], dt.bfloat16, tag="wv")
            nc.scalar.dma_start(wv_sb[:], wv_d[:])
            wo_sb = cpool.tile([P, 2, C], dt.bfloat16, tag="wo")
            nc.scalar.dma_start(wo_sb[:], wo_d[:])
            xts = []
            for ti in range(NT):
                xs = xtpool.tile([P, NCC, TCH], dt.bfloat16, tag="xt",
                                 name=f"xt{ti}")
                eng = nc.sync if ti % 2 == 0 else nc.scalar
                eng.dma_start(
                    xs[:], xt_d[:, ti * NCC * TCH:(ti + 1) * NCC * TCH])
                xts.append(xs)

            Kt = ktpool.tile([P, T], dt.bfloat16, tag="kt")
            Qt = [qtpool.tile([P, T], dt.bfloat16, tag="qt", name=f"qt{i}")
                  for i in range(2)]
            Vb = vpool.tile([P, NS, D + 1], dt.bfloat16, tag="v")
            nc.gpsimd.memset(Vb[:], 1.0)
            Ot = [otpool.tile([P, T], dt.bfloat16, tag="ot", name=f"ot{i}")
                  for i in range(2)]

            def y_chunk(ti, ec):
                t0 = ti * TCH
                y_ps = mmps.tile([P, TCH], dt.float32, tag="mm")
                for dc in range(2):
                    nc.tensor.matmul(
                        y_ps[:], wo_sb[:, dc, ec * P:(ec + 1) * P],
                        Ot[dc][:, t0:t0 + TCH],
                        start=(dc == 0), stop=(dc == 1),
                    )
                y_sb = ypool.tile([P, TCH], dt.bfloat16, tag="y")
                nc.vector.tensor_copy(y_sb[:], y_ps[:])
                blk = (ti * 8 + ec) * P
                nc.sync.dma_start(yt[blk:blk + P, :], y_sb[:])

            for ti in range(NT):
                t0 = ti * TCH
                # ---- K^T chunk (dup'd across partition halves) ----
                ps = mmps.tile([P, TCH], dt.float32, tag="mm")
                for a in range(NCC):
                    nc.tensor.matmul(
                        ps[:], wk_sb[:, a, :], xts[ti][:, a, :],
                        start=(a == 0), stop=(a == NCC - 1),
                    )
                nc.vector.tensor_scalar_add(
                    Kt[:, t0:t0 + TCH], ps[:], bk_sb[:, 0:1])
                # ---- Q^T chunk (before V: scores need Q first) ----
                for qc in range(2):
                    psq = mmps.tile([P, TCH], dt.float32, tag="mm")
                    for a in range(NCC):
                        nc.tensor.matmul(
                            psq[:], wq_sb[:, a, qc * P:(qc + 1) * P],
                            xts[ti][:, a, :],
                            start=(a == 0), stop=(a == NCC - 1),
                        )
                    nc.vector.tensor_scalar_add(
                        Qt[qc][:, t0:t0 + TCH], psq[:], bq_sb[:, qc:qc + 1])
                # ---- V chunks (natural [s, d]) ----
                for sj in range(4):
                    si = ti * 4 + sj
                    psv = mmps.tile([P, TCH], dt.float32, tag="mm")
                    for a in range(NCC):
                        nc.tensor.matmul(
                            psv[:, 0:D],
                            xts[ti][:, a, sj * P:(sj + 1) * P],
                            wv_sb[:, a, :],
                            start=(a == 0), stop=(a == NCC - 1),
                        )
                    nc.vector.tensor_add(Vb[:, si, 0:D], psv[:, 0:D],
                                         bvr_sb[:])

                # ---- attention for this t-chunk (software-pipelined:
                # scores for group i+1 are emitted before exp/PV of group
                # i so the PE streams while ACT works) ----
                nsb = (t0 + TCH) // P
                jobs = []
                for hi, h in enumerate((1, 0, 3, 2)):
                    sb = 0
                    while sb < nsb:
                        g = min(GRP, nsb - sb)
                        jobs.append((hi, h, sb, g))
                        sb += g
                o_tiles = {}

                def emit_s(job):
                    hi, h, sb, g = job
                    qc, qr = divmod(h, 2)
                    q_ap = Qt[qc][qr * D:(qr + 1) * D, t0:t0 + TCH]
                    # prefix-prune: cols < j0 of the group's FIRST block
                    # are never exp'd nor consumed
                    off = max(sb * P - t0, 0)
                    sgrp = spool.tile([P, GRP * TCH], dt.float32, tag="s")
                    for j in range(g):
                        s0 = (sb + j) * P
                        diag = s0 >= t0
                        o2 = off if j == 0 else 0
                        nc.tensor.matmul(
                            sgrp[:, j * TCH + o2:(j + 1) * TCH],
                            Kt[qr * D:(qr + 1) * D, s0:s0 + P],
                            q_ap[:, o2:],
                            start=True, stop=not diag,
                            skip_group_check=diag,
                        )
                        if diag:
                            j0 = s0 - t0
                            nc.tensor.matmul(
                                sgrp[:, j * TCH + j0:j * TCH + j0 + P],
                                ident[:], tri_sb[:],
                                start=False, stop=True,
                                skip_group_check=True,
                            )
                    return sgrp

                def emit_exp_pv(job, sgrp):
                    hi, h, sb, g = job
                    qc, qr = divmod(h, 2)
                    off = max(sb * P - t0, 0)
                    pt = ppool.tile([P, GRP * TCH], dt.bfloat16, tag="p")
                    nc.scalar.activation(
                        pt[:, off:g * TCH], sgrp[:, off:g * TCH],
                        AF.Exp, scale=0.125)
                    if sb == 0:
                        o_tiles[h] = ops_.tile([D + 1, TCH], dt.float32,
                                               tag="o", name=f"o{h}")
                    o_ps = o_tiles[h]
                    for j in range(g):
                        s0 = (sb + j) * P
                        j0 = max(s0 - t0, 0)
                        nc.tensor.matmul(
                            o_ps[:, j0:], Vb[:, sb + j, :],
                            pt[:, j * TCH + j0:(j + 1) * TCH],
                            start=(sb + j == 0),
                            stop=(sb + j == nsb - 1),
                        )
                    if sb + g == nsb:
                        finish_head(hi, h, o_ps)

                def finish_head(hi, h, o_ps):
                    qc, qr = divmod(h, 2)
                    rs = rpool.tile([1, TCH], dt.float32, tag="rs")
                    nc.vector.tensor_copy(rs[:], o_ps[D:D + 1, :])
                    rr = rpool.tile([1, TCH], dt.float32, tag="rr")
                    nc.vector.reciprocal_approx_fast(rr[:], rs[:])
                    rb_sb = rbspool.tile([D, TCH], dt.float32, tag="rbs")
                    nc.gpsimd.partition_broadcast(rb_sb[:], rr[:])
                    if qr == 0:
                        nc.vector.tensor_mul(
                            Ot[qc][0:D, t0:t0 + TCH], o_ps[0:D, :], rb_sb[:])
                    else:
                        ott = otmp.tile([D, TCH], dt.bfloat16, tag="ott")
                        nc.vector.tensor_mul(ott[:], o_ps[0:D, :], rb_sb[:])
                        nc.sync.dma_start(
                            Ot[qc][D:2 * D, t0:t0 + TCH], ott[:])
                    # weave previous chunk's output projection between heads
                    if ti > 0:
                        y_chunk(ti - 1, 2 * hi)
                        y_chunk(ti - 1, 2 * hi + 1)

                pending = emit_s(jobs[0])
                for i, job in enumerate(jobs):
                    nxt = emit_s(jobs[i + 1]) if i + 1 < len(jobs) else None
                    emit_exp_pv(job, pending)
                    pending = nxt
            for ec in range(8):
                y_chunk(NT - 1, ec)

    nc.compile()
    return nc


def get_nc():
    if "nc" not in _CACHE:
        _CACHE["nc"] = _build_nc()
    return _CACHE["nc"]


def _tile_pm(arr, ncc=NCC):
    """[ncc*128, F] -> partition-major [128, ncc*F] (contiguous rows)."""
    n, f = arr.shape
    assert n == ncc * P
    return np.ascontiguousarray(
        arr.reshape(ncc, P, f).transpose(1, 0, 2).reshape(P, ncc * f))


def make_in_maps(x, w_q, b_q, w_k, b_k, w_v, b_v, w_o, b_o):
    """Host-side sharding: per-core input maps for cores 0..7."""
    tri = np.where(
        np.arange(P)[:, None] > np.arange(P)[None, :], -1e9, 0.0
    ).astype(BF16)
    ident = np.eye(P, dtype=np.float32).astype(BF16)
    in_maps = []
    xts = {}
    for b in range(B):
        # x[b]^T tiled: cols ordered (ti, a, t)
        xT = np.ascontiguousarray(np.asarray(x[b]).T).astype(BF16)  # [C, T]
        xts[b] = np.ascontiguousarray(
            xT.reshape(NCC, P, NT, TCH).transpose(1, 2, 0, 3).reshape(
                P, NT * NCC * TCH))
    for c in range(8):
        b, kv = divmod(c, NKV)
        q0 = kv * QD
        wkk = np.concatenate([w_k[:, kv * D:(kv + 1) * D]] * 2, axis=1)
        in_maps.append({
            "xt": xts[b],
            "wq": _tile_pm(np.asarray(w_q[:, q0:q0 + QD]).astype(BF16)),
            "wk": _tile_pm(np.asarray(wkk).astype(BF16)),
            "wv": _tile_pm(
                np.asarray(w_v[:, kv * D:(kv + 1) * D]).astype(BF16)),
            "wo": np.ascontiguousarray(
                np.asarray(w_o[q0:q0 + QD, :]).astype(BF16).reshape(
                    2, P, C).transpose(1, 0, 2).reshape(P, 2 * C)),
            "bq": np.ascontiguousarray(
                np.asarray(b_q[q0:q0 + QD]).astype(np.float32).reshape(
                    2, P).T),
            "bk": np.tile(np.asarray(b_k[kv * D:(kv + 1) * D]).astype(
                np.float32), 2).reshape(P, 1),
            "bvr": np.tile(np.asarray(b_v[kv * D:(kv + 1) * D]).astype(
                np.float32)[None, :], (P, 1)),
            "tri": tri,
            "idin": ident,
        })
    return in_maps


def unshard(results, b_o):
    """Sum per-core partial outputs into the full [B, T, C] fp32 output."""
    out = np.zeros((B, T, C), np.float32)
    for c in range(8):
        b = c // NKV
        arr = results[c]["yt"].astype(np.float32)  # [NT*8*128, 512]
        # rows blk=(ti*8+ec)*128+p hold y^T[ec*128+p, ti*512+t]
        ypart_t = arr.reshape(NT, C, TCH).transpose(1, 0, 2).reshape(C, T)
        out[b] += ypart_t.T
    out += np.asarray(b_o).astype(np.float32)[None, None, :]
    return out


def kernel(x, w_q, b_q, w_k, b_k, w_v, b_v, w_o, b_o):
    from concourse.bass_utils import run_bass_kernel_spmd

    x = np.asarray(x)
    nc = get_nc()
    in_maps = make_in_maps(x, np.asarray(w_q), np.asarray(b_q),
                           np.asarray(w_k), np.asarray(b_k),
                           np.asarray(w_v), np.asarray(b_v),
                           np.asarray(w_o), np.asarray(b_o))
    res = run_bass_kernel_spmd(nc, in_maps, list(range(8)))
    return unshard(res.results, b_o)
